# revision 33
# baseline (speedup 1.0000x reference)
"""DeepSeek-V3 style MoE gate (nn_Gate) for Trainium2, 8-core data-parallel.

fp16-main + fp8-DoubleRow-correction scheme (exact to ~2^-15):
  logits = wa16 . xa16            (fp16 x fp16: 11-bit operands, exact)
         + [w8 . xl8 + wb8 . xa8] / 4096   (fp8 DoubleRow, both scaled 2^12)
where xa16 = fp16(x), xl8 = fp8((x - xa16)*4096), xa8 = fp8(xa16),
      wa16 = fp16(w), wb8 = fp8((w - wa16)*4096), w8 = fp8(w).

The matmul phase is PE-ALU-bound (fp16 FD512 MM ~517 cyc + DR FD512 MM
~542 cyc, LDWEIGHTS fully hidden by the HW pull-ahead): ~98 us/iter at
2.4 GHz is the floor for this numeric scheme, and the scheme itself is
forced - numpy simulation shows any cheaper decomposition (bf16, fp16-
only, single correction term, or a partially dropped wb8 term) flips too
many near-tie top-8 selections to pass the 2e-2 gate.

Structural changes vs the original baseline:
  - x planes stream in 1 MB chunks (4 k-pairs per dma_start).
  - sigmoid + expert-bias add run on ACT in expert-major layout (bias is
    a per-partition scalar there); the PE transposes u = sigmoid(lg)+bias
    and the DVE chain reads the PSUM-resident transposed tile directly.
  - top-8 chain uses a bias-perturbation trick: extract max8 of u_m and
    of u_m + bias*2^-13; the scaled difference recovers bias at the
    selected experts, so weights = fvals - 8192*(fvalsb - fvals) with no
    second extraction / alignment pass (numpy-validated: metric identical
    to the exact chain, weight rel err 2.2e-4).  Renorm math is batched
    [128,32] per 512-token block.
  - outputs packed in SBUF; one wout/iout DMA pair per 512-token block,
    indices written by max_index as uint32 and bitcast to int32.
  - per-block epilogues are emitted after the NEXT block's first matmul
    group so their DVE chains overlap the following matmul phase.
  - weight preamble sliced; in the single-shot build the slices trickle
    between x-chunk DMAs one group ahead of use, and a few dummy warmup
    matmuls un-throttle the PE HAM clock during the preamble.
  - reps timing loop unrolls 8 iterations per hardware-loop body to
    amortize the loop-boundary engine drain (tc.For_i quiesces all
    engines each loop iteration, serializing the last top-8 chain tail).
"""
import numpy as np
import ml_dtypes
import concourse.bass as bass
import concourse.tile as tile
from concourse import bacc, mybir
from concourse.masks import make_identity
from concourse.bass_utils import run_bass_kernel_spmd

AOT = mybir.AluOpType
F32 = mybir.dt.float32
F16 = mybir.dt.float16
FP8 = mybir.dt.float8e4
U16 = mybir.dt.uint16
I32 = mybir.dt.int32
DR = mybir.MatmulPerfMode.DoubleRow
ACTF = mybir.ActivationFunctionType

N_TOKENS = 8192
K = 7168
NK = K // 128          # 56 fp16 k-tiles
NKK = K // 256         # 28 k-pair tiles
NG = NKK // 4          # 7 groups of 4 k-pairs (1 MB DMA chunks)
E = 256
N_CORES = 8
T_CORE = N_TOKENS // N_CORES   # 1024
TB = 512
NTB = T_CORE // TB             # 2
S = 4096.0
DELTA = 2.0 ** -13             # bias perturbation for weight recovery


def _chain_wide(nc, pool, tp, biasd_b, fvals_all, fvalsb_all, iout_u32, t8, col):
    """Per-128-token-tile selection (wide DVE ops only).

    tp: [128,256] f32 PSUM tile holding u = sigmoid(logits) + bias,
    token-major.  Writes top-8 values of u_m and of u_m + bias*DELTA into
    column slices of the per-TB packed tiles, and indices (u32) into the
    packed index tile.
    """
    tp3 = tp[:, 0:256].rearrange("p (g e) -> p g e", g=8)
    # top-2 per group of 32: group max, zap it, group max again
    gmax1 = pool.tile([128, 8], F32, tag="gmax1")
    nc.vector.tensor_reduce(gmax1[:], tp3, axis=mybir.AxisListType.X, op=AOT.max)
    u_z = pool.tile([128, 256], F32, tag="u_z")
    nc.vector.match_replace(u_z[:], gmax1[:], tp[:, 0:256], -1e30)
    gmax2 = pool.tile([128, 8], F32, tag="gmax2")
    nc.vector.tensor_reduce(gmax2[:], u_z[:].rearrange("p (g e) -> p g e", g=8),
                            axis=mybir.AxisListType.X, op=AOT.max)
    g2sum = pool.tile([128, 8], F32, tag="g2sum")
    nc.vector.tensor_add(g2sum[:], gmax1[:], gmax2[:])
    # top-4 groups: threshold at 4th largest group score
    gtop = pool.tile([128, 8], F32, tag="gtop")
    nc.vector.max(out=gtop[:], in_=g2sum[:])
    gmask = pool.tile([128, 8], F32, tag="gmask")
    nc.vector.tensor_scalar(gmask[:], g2sum[:], gtop[:, 3:4], None, op0=AOT.is_ge)
    # mask = multiply by 0/1 exactly like the reference
    u_m = pool.tile([128, 256], F32, tag="u_m")
    nc.vector.tensor_tensor(
        out=u_m[:].rearrange("p (g e) -> p g e", g=8),
        in0=tp3,
        in1=gmask[:].unsqueeze(-1).to_broadcast([128, 8, 32]),
        op=AOT.mult,
    )
    u_b = pool.tile([128, 256], F32, tag="u_b")
    nc.vector.tensor_add(u_b[:], u_m[:], biasd_b[:])
    nc.vector.max(out=fvals_all[:, col * 8:(col + 1) * 8], in_=u_m[:])
    nc.vector.max_index(iout_u32[:, t8 * 8:(t8 + 1) * 8],
                        fvals_all[:, col * 8:(col + 1) * 8], u_m[:])
    nc.vector.max(out=fvalsb_all[:, col * 8:(col + 1) * 8], in_=u_b[:])


def _chain_tail(nc, pool, fvals_all, fvalsb_all, out_w, tb):
    """TB-batched weight recovery + renorm over 4 tiles at once ([128,32])."""
    tdiff = pool.tile([128, 32], F32, tag="tdiff")
    nc.vector.tensor_sub(tdiff[:], fvalsb_all[:], fvals_all[:])
    ssel = pool.tile([128, 32], F32, tag="ssel")
    nc.vector.scalar_tensor_tensor(
        out=ssel[:], in0=tdiff[:], scalar=-1.0 / DELTA, in1=fvals_all[:],
        op0=AOT.mult, op1=AOT.add)
    denom = pool.tile([128, 4], F32, tag="denom")
    nc.vector.tensor_reduce(denom[:], ssel[:].rearrange("p (t e) -> p t e", t=4),
                            axis=mybir.AxisListType.X, op=AOT.add)
    recip = pool.tile([128, 4], F32, tag="recip")
    nc.vector.reciprocal(recip[:], denom[:])
    rs = pool.tile([128, 4], F32, tag="rs")
    nc.vector.tensor_scalar(rs[:], recip[:], 2.5, None, op0=AOT.mult)
    nc.vector.tensor_tensor(
        out=out_w[:, tb * 32:(tb + 1) * 32].rearrange("p (t e) -> p t e", t=4),
        in0=ssel[:].rearrange("p (t e) -> p t e", t=4),
        in1=rs[:].unsqueeze(-1).to_broadcast([128, 4, 8]),
        op=AOT.mult,
    )


def build_kernel(reps=None, sw_interleave=False):
    nc = bacc.Bacc("TRN2", target_bir_lowering=False, debug=False,
                   enable_asserts=False, num_devices=N_CORES)
    # host-prepacked x planes; per (tb, g) one [128, 4096] f16 row-block and
    # one [128, 8192] fp8 row-block (1 MB DMA each, 8 KB contiguous rows).
    xa_in = nc.dram_tensor("xa_p", [NTB * NG * 128, 4096], F16,
                           kind="ExternalInput").ap()
    x8_in = nc.dram_tensor("x8_p", [NTB * NG * 128, 8192], FP8,
                           kind="ExternalInput").ap()
    wa_in = nc.dram_tensor("wa", [K, E], F16, kind="ExternalInput").ap()
    w8_in = nc.dram_tensor("w8", [K, E], FP8, kind="ExternalInput").ap()
    wb8_in = nc.dram_tensor("wb8", [K, E], FP8, kind="ExternalInput").ap()
    biasd_in = nc.dram_tensor("biasdb", [128, E], F32, kind="ExternalInput").ap()
    biaspp_in = nc.dram_tensor("biaspp", [128, 2], F32, kind="ExternalInput").ap()
    wout = nc.dram_tensor("wout", [T_CORE, 8], F32, kind="ExternalOutput").ap()
    iout = nc.dram_tensor("iout", [T_CORE, 8], I32, kind="ExternalOutput").ap()

    import contextlib
    with tile.TileContext(nc) as tc:
        with (
            tc.tile_pool(name="wres", bufs=1) as wres,
            tc.tile_pool(name="consts", bufs=1) as consts,
            tc.tile_pool(name="xs", bufs=4) as xs,
            tc.tile_pool(name="x8s", bufs=4) as x8s,
            tc.tile_pool(name="mmps", bufs=2, space="PSUM") as mmps,
            tc.tile_pool(name="tps", bufs=3, space="PSUM") as tps,
            tc.tile_pool(name="warmp", bufs=1, space="PSUM") as warmp,
            tc.tile_pool(name="sig", bufs=2) as sigp,
            tc.tile_pool(name="chain", bufs=3) as chain,
            tc.tile_pool(name="outs", bufs=1) as outsp,
        ):
            # --- weight / const preamble, sliced and interleaved so the k=0
            # matmuls (fp16 AND DR) can start after ~1/4 of the load; fp8
            # slices ride the scalar queue in parallel with wa on sync ---
            wa_t = wres.tile([128, NK * E], F16, tag="wa_t")
            wa_r = wa_in.rearrange("(nk p) e -> p nk e", p=128)
            w8_t = wres.tile([128, NKK * 2 * E], FP8, tag="w8_t")
            w8_r = w8_in.rearrange("(nkk two p) e -> p nkk two e", two=2, p=128)
            wb8_t = wres.tile([128, NKK * 2 * E], FP8, tag="wb8_t")
            wb8_r = wb8_in.rearrange("(nkk two p) e -> p nkk two e", two=2, p=128)
            w8_v = w8_t[:].rearrange("p (nkk two e) -> p nkk two e", two=2, e=E)
            wb8_v = wb8_t[:].rearrange("p (nkk two e) -> p nkk two e", two=2, e=E)
            def emit_wslice(s):
                sl = slice(s * 7, (s + 1) * 7)
                nc.sync.dma_start(
                    wa_t[:].rearrange("p (nk e) -> p nk e", e=E)[:, sl],
                    wa_r[:, sl])
                if s < 4:
                    sl8 = slice(s * 7, (s + 1) * 7)
                    nc.scalar.dma_start(w8_v[:, sl8], w8_r[:, sl8])
                    nc.scalar.dma_start(wb8_v[:, sl8], wb8_r[:, sl8])

            emit_wslice(0)
            wslices_left = list(range(1, 8))
            if reps:
                # reps mode: the loop body repeats, so all weight loads must
                # sit ahead of it; first-body queueing cost is amortized.
                for s in wslices_left:
                    emit_wslice(s)
                wslices_left = []
            biasd_b = consts.tile([128, E], F32, tag="biasd_b")
            nc.sync.dma_start(biasd_b[:], biasd_in[:])
            bias_pp = consts.tile([128, 2], F32, tag="bias_pp")
            nc.sync.dma_start(bias_pp[:], biaspp_in[:])
            ident = consts.tile([128, 128], F32, tag="ident")
            make_identity(nc, ident[:])
            # packed outputs
            out_w = outsp.tile([128, 8 * NTB * 4], F32, tag="out_w")
            out_iu = outsp.tile([128, 8 * NTB * 4], mybir.dt.uint32, tag="out_iu")

            # HAM warmup: dense fp32 matmuls during the preamble DMAs (long
            # FD so the LDWEIGHTS fraction stays small and the activity
            # window reads busy)
            warm = warmp.tile([128, 512], F32, tag="warm")
            wsrc0 = consts.tile([128, 448], F32, tag="wsrc0")
            nc.scalar.memzero(wsrc0[:])
            for _ in range(6):
                nc.tensor.matmul(warm[:, 0:448], ident[:], wsrc0[:],
                                 start=True, stop=True)

            w8v = w8_t[:].rearrange("p (nkk two e) -> p nkk two e", two=2, e=E)
            wb8v = wb8_t[:].rearrange("p (nkk two e) -> p nkk two e", two=2, e=E)
            wav = wa_t[:].rearrange("p (nk e) -> p nk e", e=E)

            def matmul_phase(tb):
                psA = [mmps.tile([128, TB], F32, tag="psA", name=f"psA_{tb}_{i}")
                       for i in range(2)]
                psB = [mmps.tile([128, TB], F32, tag="psB", name=f"psB_{tb}_{i}")
                       for i in range(2)]
                first_group_done = None
                for g in range(NG):
                    if tb == 0:
                        # single-shot: trickle remaining weight slices one
                        # group ahead of their first use, interleaved with the
                        # x-chunk DMAs so neither starves the other
                        while (wslices_left
                               and wslices_left[0] * 7 <= 8 * g + 15):
                            emit_wslice(wslices_left.pop(0))
                    base = (tb * NG + g) * 128
                    xa_g = xs.tile([128, 4096], F16, tag="xa_g")
                    x8_g = x8s.tile([128, 8192], FP8, tag="x8_g")
                    if g == 0:
                        # split the first chunk so post-barrier matmuls can
                        # start after half the transfer
                        nc.sync.dma_start(xa_g[:, 0:2048], xa_in[base:base + 128, 0:2048])
                        nc.sync.dma_start(xa_g[:, 2048:4096], xa_in[base:base + 128, 2048:4096])
                        nc.scalar.dma_start(x8_g[:, 0:4096], x8_in[base:base + 128, 0:4096])
                        nc.scalar.dma_start(x8_g[:, 4096:8192], x8_in[base:base + 128, 4096:8192])
                    else:
                        nc.sync.dma_start(xa_g[:], xa_in[base:base + 128, :])
                        nc.scalar.dma_start(x8_g[:], x8_in[base:base + 128, :])
                    # last group: all eh0 matmuls first so the eh0 PSUM banks
                    # close early and the ACT epilogue overlaps eh1's matmuls
                    eh_major = (g == NG - 1)
                    for eh_o in range(2 if eh_major else 1):
                        for kk4 in range(4):
                            kk = g * 4 + kk4
                            for i in range(2):
                                k = 2 * kk + i
                                xsl = xa_g[:, (kk4 * 2 + i) * TB:(kk4 * 2 + i + 1) * TB]
                                x8b = x8_g[:, kk4 * 2048 + i * 1024:
                                           kk4 * 2048 + (i + 1) * 1024]
                                wsrc = w8v if i == 0 else wb8v
                                ehs = [eh_o] if eh_major else [0, 1]
                                for eh in ehs:
                                    nc.tensor.matmul(
                                        psA[eh][:], wav[:, k, eh * 128:(eh + 1) * 128],
                                        xsl, start=(k == 0), stop=(k == NK - 1))
                                    nc.tensor.matmul(
                                        psB[eh][:],
                                        wsrc[:, kk, :, eh * 128:(eh + 1) * 128],
                                        x8b.rearrange("p (two t) -> p two t", two=2),
                                        start=(kk == 0 and i == 0),
                                        stop=(kk == NKK - 1 and i == 1),
                                        perf_mode=DR, skip_group_check=True)
                    if g == 0 and matmul_phase.pending is not None:
                        matmul_phase.pending()
                        matmul_phase.pending = None
                return psA, psB

            matmul_phase.pending = None

            def epilogue(tb, psA, psB):
                # expert-major: corr (ACT, PSUM->SBUF), lg = corr + psA (DVE),
                # u = sigmoid(lg) (ACT), u += bias[e] (ACT per-partition);
                # then PE transposes u per 128-token column tile and the DVE
                # chain runs on the PSUM-resident transposed tile.
                uTb = [None, None]
                for eh in range(2):
                    corr = sigp.tile([128, TB], F32, tag="corr",
                                     name=f"corr_{tb}_{eh}")
                    nc.scalar.mul(corr[:], psB[eh][:], 1.0 / S)
                    lg = sigp.tile([128, TB], F32, tag="lg", name=f"lg_{tb}_{eh}")
                    nc.vector.tensor_add(lg[:], corr[:], psA[eh][:])
                    # sigmoid + per-expert bias add in half-width chunks so
                    # the first transposes start one chunk earlier
                    uT = sigp.tile([128, TB], F32, tag="uT", name=f"uT_{tb}_{eh}")
                    uTb[eh] = sigp.tile([128, TB], F32, tag="uTb",
                                        name=f"uTb_{tb}_{eh}")
                    for h in range(2):
                        hs = slice(h * (TB // 2), (h + 1) * (TB // 2))
                        nc.scalar.activation(uT[:, hs], lg[:, hs], ACTF.Sigmoid)
                        nc.scalar.activation(uTb[eh][:, hs], uT[:, hs],
                                             ACTF.Identity,
                                             bias=bias_pp[:, eh:eh + 1])
                fvals_all = chain.tile([128, 32], F32, tag="fvals_all",
                                       name=f"fva_{tb}")
                fvalsb_all = chain.tile([128, 32], F32, tag="fvalsb_all",
                                        name=f"fvb_{tb}")
                for col in range(4):
                    t8 = tb * 4 + col
                    tp = tps.tile([128, 512], F32, tag="tp", name=f"tp_{t8}")
                    for eh in range(2):
                        nc.tensor.transpose(
                            tp[:, eh * 128:(eh + 1) * 128],
                            uTb[eh][:, col * 128:(col + 1) * 128], ident[:])
                    _chain_wide(nc, chain, tp, biasd_b, fvals_all, fvalsb_all,
                                out_iu, t8, col)
                _chain_tail(nc, chain, fvals_all, fvalsb_all, out_w, tb)
                # stream this TB's outputs out now (tail pays only last TB's)
                tsl = slice(tb * 4, (tb + 1) * 4)
                nc.sync.dma_start(
                    wout.rearrange("(t p) e -> p t e", p=128)[:, tsl],
                    out_w[:, tb * 32:(tb + 1) * 32].rearrange(
                        "p (t e) -> p t e", e=8))
                nc.sync.dma_start(
                    iout.rearrange("(t p) e -> p t e", p=128)[:, tsl],
                    out_iu[:, tb * 32:(tb + 1) * 32].bitcast(I32).rearrange(
                        "p (t e) -> p t e", e=8))

            unroll = 8 if reps else 1
            loop_ctx = (tc.For_i(0, (reps + unroll - 1) // unroll, 1)
                        if reps else contextlib.nullcontext())
            with loop_ctx:
                for _u in range(unroll):
                    for tb in range(NTB):
                        psA, psB = matmul_phase(tb)
                        matmul_phase.pending = (
                            lambda tb=tb, psA=psA, psB=psB: epilogue(tb, psA, psB))
                # last epilogue of the body (others were deferred into the
                # following matmul phase so its PE work overlaps their chains)
                matmul_phase.pending()
                matmul_phase.pending = None
    nc.compile()
    return nc


def host_prep(x, weight, bias):
    x = np.ascontiguousarray(np.asarray(x, dtype=np.float32))
    weight = np.ascontiguousarray(np.asarray(weight, dtype=np.float32))
    bias = np.asarray(bias, dtype=np.float32)
    f16 = np.float16
    f8 = ml_dtypes.float8_e4m3

    wa = weight.astype(f16)
    wb8 = ((weight - wa.astype(np.float32)) * S).astype(f8)
    w8 = weight.astype(f8)
    waT = np.ascontiguousarray(wa.T)
    w8T = np.ascontiguousarray(w8.T)
    wb8T = np.ascontiguousarray(wb8.T)
    biasdb = np.ascontiguousarray(
        np.broadcast_to(bias * np.float32(DELTA), (128, E))).astype(np.float32)
    biaspp = np.ascontiguousarray(bias.reshape(2, 128).T)  # [128, eh]

    xa_all = x.astype(f16)
    xl8_all = ((x - xa_all.astype(np.float32)) * S).astype(f8)
    xa8_all = xa_all.astype(f8)

    def pack_xa(aT):
        # aT [K, T_CORE] -> rows (tb, g, p), cols (kk4, i, t)
        a6 = aT.reshape(NG, 4, 2, 128, NTB, TB)        # g kk4 i p tb t
        return np.ascontiguousarray(
            a6.transpose(4, 0, 3, 1, 2, 5).reshape(NTB * NG * 128, 4096))

    def pack_x8(lT, aT):
        # cols per kk4: [xl8 i0 t | xl8 i1 t | xa8 i0 t | xa8 i1 t]
        l6 = lT.reshape(NG, 4, 2, 128, NTB, TB).transpose(4, 0, 3, 1, 2, 5)
        a6 = aT.reshape(NG, 4, 2, 128, NTB, TB).transpose(4, 0, 3, 1, 2, 5)
        # -> [tb, g, p, kk4, i, t]; stack plane axis after kk4
        both = np.stack([l6, a6], axis=4)              # tb g p kk4 plane i t
        return np.ascontiguousarray(both.reshape(NTB * NG * 128, 8192))

    in_maps = []
    for c in range(N_CORES):
        sl = slice(c * T_CORE, (c + 1) * T_CORE)
        xa_p = pack_xa(np.ascontiguousarray(xa_all[sl].T))
        x8_p = pack_x8(np.ascontiguousarray(xl8_all[sl].T),
                       np.ascontiguousarray(xa8_all[sl].T))
        in_maps.append({
            "xa_p": xa_p,
            "x8_p": x8_p,
            "wa": waT,
            "w8": w8T,
            "wb8": wb8T,
            "biasdb": biasdb,
            "biaspp": biaspp,
        })
    return in_maps


_CACHED = {}


def kernel(x, token_mask, weight, bias):
    in_maps = host_prep(x, weight, bias)
    if "nc" not in _CACHED:
        _CACHED["nc"] = build_kernel()
    nc = _CACHED["nc"]
    res = run_bass_kernel_spmd(nc, in_maps, core_ids=list(range(N_CORES)))
    weights_full = np.concatenate([r["wout"] for r in res.results], axis=0)
    idx_full = np.concatenate([r["iout"] for r in res.results], axis=0)
    return weights_full.astype(np.float32), idx_full.astype(np.int32)
